# revision 6
# baseline (speedup 1.0000x reference)
"""Causal self-attention with RoPE for TRN2.

This environment executes kernels through a tunneled PJRT path whose
per-exec cost is dominated by (a) ~70ms fixed dispatch overhead and
(b) ~0.12ms per aggregate MiB of ExternalInput/Output buffers shipped
across the tunnel every execution. Real device compute (~3ms here) is
almost free by comparison, and on-device collectives bounce through the
same tunnel, so multi-core sharding only multiplies shipped bytes
(the old 8-core version shipped x replicated 8x + 8 fp32 partial
outputs ~= 595 MiB aggregate -> ~138ms).

So: run the WHOLE problem on ONE core and ship everything once, in
fp16 (0.05% rounding - comfortably inside the 2e-2 gate):
  x 16 MiB + Wq/Wk/Wv/Wo 32 MiB + out 16 MiB + consts ~1.3 MiB
  ~= 65 MiB aggregate -> ~8ms shipping + ~70ms fixed + ~3ms compute.

Kernel structure (per batch):
  - 8 head-pairs: QKV projections (fp16 matmuls, f32 PSUM accum),
    RoPE via a rotate-half permutation matmul + cos/sin vector ops,
    per-head causal attention with transposed layouts
    (qT/kT/vT: [hd,S], scoresT/attnT: [j,q]) and no-max softmax
    (scaled scores ~N(0,1), exp is safe; sums via ones-column matmul,
    1/sum broadcast via K=1 ones-row matmul).
  - attnout for all 16 heads kept in SBUF as fp16 [128, 16, S],
    then the output projection accumulates all heads in PSUM.
"""
import sys

sys.path.insert(0, "/opt/trn_rl_repo")

import numpy as np

import concourse.bass as bass
import concourse.bacc as bacc
import concourse.mybir as mybir
import concourse.tile as tile
from concourse.bass_utils import run_bass_kernel_spmd

F32 = mybir.dt.float32
F16 = mybir.dt.float16

B, S, D, H, HD = 2, 2048, 2048, 16, 128
N_CORES = 1
HPC = 2                     # heads per pair-iteration
FPP = HPC * HD              # features per pair = 256
NPAIR = H // HPC            # 8 head-pairs
SCALE = 1.0 / float(np.sqrt(HD))
NKT = D // 128              # 16 contraction tiles
NSC = S // 512              # 4 s-chunks per batch
NJT = S // 128              # 16 key tiles per batch
BS = B * S
# packed const layout: [cos | sin | mask | rmat | ident]
CO, SO, MO, RO, IO = 0, S, 2 * S, 2 * S + 896, 2 * S + 896 + 128
CW = IO + 128
CWPAD = 3 * 2048            # cst padded to a multiple of D columns
MROWS = 4096 + 4 * D + (128 * CWPAD) // D


def _projections(nc, P, b, hp):
    """QKV projections for head-pair hp of batch b -> qTs/kTs/vTs (fp16)."""
    fs = slice(FPP * hp, FPP * hp + FPP)
    wq_g, wk_g, wv_g = [], [], []
    for g in range(NKT // 4):
        wqt = P.wtp.tile([128, 4, FPP], F16, name=f"wq{g}", tag=f"wq{g}")
        wkt = P.wtp.tile([128, 4, FPP], F16, name=f"wk{g}", tag=f"wk{g}")
        wvt = P.wtp.tile([128, 4, FPP], F16, name=f"wv{g}", tag=f"wv{g}")
        nc.scalar.dma_start(wqt[:], P.wq_r[g, :, :, fs])
        nc.scalar.dma_start(wkt[:], P.wk_r[g, :, :, fs])
        nc.scalar.dma_start(wvt[:], P.wv_r[g, :, :, fs])
        wq_g.append(wqt); wk_g.append(wkt); wv_g.append(wvt)
    qTs, kTs, vTs = [], [], []
    for h in range(HPC):
        qTs.append(P.qkvp.tile([128, S], F16, name=f"qT{h}", tag=f"qT{h}"))
        kTs.append(P.qkvp.tile([128, S], F16, name=f"kT{h}", tag=f"kT{h}"))
        vTs.append(P.qkvp.tile([128, S], F16, name=f"vT{h}", tag=f"vT{h}"))
    for sc in range(NSC):
        ss = slice(512 * sc, 512 * sc + 512)
        acc = [P.paccp.tile([128, 512], F32, name=f"pa{_j}", tag="pacc") for _j in range(6)]
        for g in range(NKT // 4):
            xt = P.xtp.tile([128, 4, 512], F16, tag="xt")
            eng = nc.sync if g % 2 == 0 else nc.gpsimd
            eng.dma_start(xt[:], P.xT_r[g, :, :, b * S + 512 * sc:
                                         b * S + 512 * sc + 512])
            _proj_mms(nc, acc, wq_g[g], wk_g[g], wv_g[g], xt, g)
        for h in range(HPC):
            nc.scalar.copy(qTs[h][:, ss], acc[h][:])
            nc.scalar.copy(kTs[h][:, ss], acc[2 + h][:])
            nc.scalar.copy(vTs[h][:, ss], acc[4 + h][:])
    return qTs, kTs, vTs


def _proj_mms(nc, acc, wqt, wkt, wvt, xt, g):
    for i in range(4):
        kt = 4 * g + i
        st, sp = kt == 0, kt == NKT - 1
        for h in range(HPC):
            hs = slice(128 * h, 128 * h + 128)
            nc.tensor.matmul(acc[h][:], wqt[:, i, hs], xt[:, i, :],
                             start=st, stop=sp)
            nc.tensor.matmul(acc[2 + h][:], wkt[:, i, hs], xt[:, i, :],
                             start=st, stop=sp)
            nc.tensor.matmul(acc[4 + h][:], wvt[:, i, hs], xt[:, i, :],
                             start=st, stop=sp)


def _rope(nc, P, t_):
    """RoPE in place on a [128, S] fp16 tile."""
    for sc in range(NSC):
        ss = slice(512 * sc, 512 * sc + 512)
        ps_rot = P.paccp.tile([128, 512], F32, tag="pacc")
        nc.tensor.matmul(ps_rot[:], P.cst_sb[:, RO:RO + 128], t_[:, ss],
                         start=True, stop=True)
        t2 = P.ropep.tile([128, 512], F16, tag="ropetmp")
        nc.vector.tensor_mul(t2[:], ps_rot[:], P.cst_sb[:, SO + 512 * sc:SO + 512 * sc + 512])
        t1 = P.ropep.tile([128, 512], F16, tag="ropetmp")
        nc.vector.tensor_mul(t1[:], t_[:, ss], P.cst_sb[:, CO + 512 * sc:CO + 512 * sc + 512])
        nc.vector.tensor_add(t_[:, ss], t1[:], t2[:])


def _attention(nc, P, o_sb, hg, qT, kT, vT):
    """Causal attention for one head; writes o_sb[:, hg, :]."""
    Exp = mybir.ActivationFunctionType.Exp
    v_h = P.vhp.tile([128, NJT, 128], F16, tag="v_h")
    for jt in range(NJT):
        js = slice(128 * jt, 128 * jt + 128)
        ps_tp = P.paccp.tile([128, 128], F16, tag="pacc")
        nc.tensor.transpose(ps_tp[:], vT[:, js], P.cst_sb[:, IO:IO + 128])
        nc.scalar.copy(v_h[:, jt, :], ps_tp[:])
    for qc in range(NSC):
        qs = slice(512 * qc, 512 * qc + 512)
        ps_av = P.pavp.tile([128, 512], F32, tag="pav")
        ps_sum = P.psum1p.tile([1, 512], F32, tag="psum1")
        njt = 4 * qc + 4
        for jt in range(njt):
            js = slice(128 * jt, 128 * jt + 128)
            ps_sc = P.paccp.tile([128, 512], F32, tag="pacc")
            nc.tensor.matmul(ps_sc[:], kT[:, js], qT[:, qs],
                             start=True, stop=True)
            at = P.attnp.tile([128, 512], F16, tag="at")
            nc.scalar.activation(at[:], ps_sc[:], Exp, scale=SCALE)
            if jt >= 4 * qc:
                mi = 384 - 128 * (jt - 4 * qc)
                nc.vector.tensor_mul(at[:], at[:], P.cst_sb[:, MO + mi:MO + mi + 512])
            st, sp = jt == 0, jt == njt - 1
            nc.tensor.matmul(ps_sum[:], P.cst_sb[:, MO + 895:MO + 896], at[:],
                             start=st, stop=sp)
            nc.tensor.matmul(ps_av[:], v_h[:, jt, :], at[:],
                             start=st, stop=sp)
        sums_sb = P.smallp.tile([1, 512], F32, tag="sums")
        nc.scalar.copy(sums_sb[:], ps_sum[:])
        recip = P.smallp.tile([1, 512], F16, tag="recip")
        nc.vector.reciprocal(recip[:], sums_sb[:])
        ps_bc = P.paccp.tile([128, 512], F32, tag="pacc")
        nc.tensor.matmul(ps_bc[:], P.cst_sb[0:1, MO + 384:MO + 512], recip[:],
                         start=True, stop=True)
        recipT = P.smallp.tile([128, 512], F32, tag="recipT")
        nc.scalar.copy(recipT[:], ps_bc[:])
        nc.vector.tensor_mul(o_sb[:, hg, qs], ps_av[:], recipT[:])


def _outproj(nc, P, o_sb, b):
    """Output projection for batch b: all 16 heads accumulated in PSUM."""
    for dt in range(D // 128):
        ds = slice(128 * dt, 128 * dt + 128)
        wo_t = P.wotp.tile([128, H, 128], F16, tag="wo_t")
        nc.gpsimd.dma_start(wo_t[:], P.wo_r[:, :, ds])
        for half in range(2):
            outt = P.outevp.tile([128, 1024], F16, tag="outt")
            for j in range(2):
                sc = 2 * half + j
                ss = slice(512 * sc, 512 * sc + 512)
                ps_o = P.paccp.tile([128, 512], F32, tag="pacc")
                for h in range(H):
                    nc.tensor.matmul(ps_o[:], wo_t[:, h, :], o_sb[:, h, ss],
                                     start=(h == 0), stop=(h == H - 1))
                nc.vector.tensor_copy(outt[:, 512 * j:512 * j + 512], ps_o[:])
            oeng = nc.sync if (dt + half) % 2 == 0 else nc.gpsimd
            oeng.dma_start(P.out_r[dt, :, b * S + 1024 * half:
                                   b * S + 1024 * half + 1024], outt[:])


class _Pools:
    pass


def build_nc():
    nc = bacc.Bacc(None, target_bir_lowering=False, debug=False)

    # single packed input: [xT (4096 rows) | wq/wk/wv/wo.T (8192) | cst (384)]
    mega_d = nc.dram_tensor("mega", [MROWS, D], F16, kind="ExternalInput")
    out_d = nc.dram_tensor("outP", [D, BS], F16, kind="ExternalOutput")

    P = _Pools()
    # group kt tiles in fours so each DMA moves a big block in one descriptor
    P.xT_r = mega_d[0:4096, :].rearrange(
        "(g t p r2) c -> g p t (r2 c)", g=4, t=4, p=128, r2=2)
    wpart = mega_d[4096:4096 + 4 * D, :]
    wAll_r = wpart.rearrange("(m g t p) f -> m g p t f", m=4, p=128, t=4)
    P.wq_r, P.wk_r, P.wv_r = wAll_r[0], wAll_r[1], wAll_r[2]
    P.wo_r = wpart.rearrange("(m h p) d -> m p h d", m=4, p=128)[3]
    P.cst_r = mega_d[4096 + 4 * D:MROWS, :].rearrange(
        "(p a) c -> p (a c)", p=128)
    P.out_r = out_d[:].rearrange("(dt p) s -> dt p s", p=128)

    with tile.TileContext(nc) as tc:
        with (
            nc.allow_low_precision(reason="fp16 pipeline is intended"),
            tc.tile_pool(name="const", bufs=1) as constp,
            tc.tile_pool(name="xt", bufs=3) as xtp,
            tc.tile_pool(name="wt", bufs=1) as wtp,
            tc.tile_pool(name="qkv", bufs=1) as qkvp,
            tc.tile_pool(name="vh", bufs=2) as vhp,
            tc.tile_pool(name="rope", bufs=2) as ropep,
            tc.tile_pool(name="attn", bufs=5) as attnp,
            tc.tile_pool(name="small", bufs=2) as smallp,
            tc.tile_pool(name="osb", bufs=1) as osbp,
            tc.tile_pool(name="wot", bufs=2) as wotp,
            tc.tile_pool(name="outev", bufs=3) as outevp,
            tc.tile_pool(name="pacc", bufs=6, space="PSUM") as paccp,
            tc.tile_pool(name="pav", bufs=1, space="PSUM") as pavp,
            tc.tile_pool(name="psum1", bufs=1, space="PSUM") as psum1p,
        ):
            P.xtp, P.wtp, P.qkvp, P.vhp = xtp, wtp, qkvp, vhp
            P.ropep, P.attnp, P.smallp, P.osbp = ropep, attnp, smallp, osbp
            P.wotp, P.outevp = wotp, outevp
            P.paccp, P.pavp, P.psum1p = paccp, pavp, psum1p

            # ---- constants: packed region of mega ----
            cst_sb = constp.tile([128, CWPAD], F16, name="cst_sb")
            nc.scalar.dma_start(cst_sb[:], P.cst_r)
            P.cst_sb = cst_sb

            for b in range(B):
                o_sb = osbp.tile([128, H, S], F16, tag="o_sb")
                for hp in range(NPAIR):
                    qTs, kTs, vTs = _projections(nc, P, b, hp)
                    for h in range(HPC):
                        _rope(nc, P, qTs[h])
                        _rope(nc, P, kTs[h])
                        _attention(nc, P, o_sb, HPC * hp + h,
                                   qTs[h], kTs[h], vTs[h])
                _outproj(nc, P, o_sb, b)

    nc.compile()
    return nc


_NC_CACHE = None


def _get_nc():
    global _NC_CACHE
    if _NC_CACHE is None:
        _NC_CACHE = build_nc()
    return _NC_CACHE


def _host_consts():
    inv_freq = 1.0 / (10000.0 ** (np.arange(0, HD, 2, dtype=np.float32) / HD))
    t = np.arange(S, dtype=np.float32)
    freqs = np.outer(t, inv_freq)
    emb = np.concatenate([freqs, freqs], axis=-1)          # [S, hd]
    cosT = np.cos(emb).T.astype(np.float16)                 # [hd, S]
    sinT = np.sin(emb).T.astype(np.float16)
    # staircase mask: variant i is the slice [:, 384-128i : 384-128i+512]
    r = np.arange(128)[:, None]
    u = np.arange(896)[None, :]
    mask = (u >= r + 384).astype(np.float16)
    rmat = np.zeros((128, 128), np.float16)
    for m in range(64):
        rmat[m + 64, m] = -1.0
        rmat[m, m + 64] = 1.0
    ident = np.eye(128, dtype=np.float16)
    return cosT, sinT, mask, rmat, ident


def _make_in_maps(inputs):
    x = np.asarray(inputs["x"], dtype=np.float32)
    Wq = np.asarray(inputs["Wq"], dtype=np.float32)
    Wk = np.asarray(inputs["Wk"], dtype=np.float32)
    Wv = np.asarray(inputs["Wv"], dtype=np.float32)
    Wo = np.asarray(inputs["Wo"], dtype=np.float32)

    xT = np.ascontiguousarray(x.reshape(BS, D).T).astype(np.float16)
    cosT, sinT, mask, rmat, ident = _host_consts()

    cst = np.zeros((128, CWPAD), np.float16)
    cst[:, :CW] = np.concatenate([cosT, sinT, mask, rmat, ident], axis=1)
    mega = np.concatenate(
        [xT.reshape(-1, D)]
        + [np.ascontiguousarray(W.T).astype(np.float16)
           for W in (Wq, Wk, Wv, Wo)]
        + [cst.reshape(-1, D)], axis=0)
    return [dict(mega=mega)]


def kernel(x, Wq, Wk, Wv, Wo):
    in_maps = _make_in_maps(dict(x=x, Wq=Wq, Wk=Wk, Wv=Wv, Wo=Wo))
    nc = _get_nc()
    res = run_bass_kernel_spmd(nc, in_maps, core_ids=list(range(N_CORES)))
    outT = res.results[0]["outP"].astype(np.float32)
    return np.ascontiguousarray(outT.T).reshape(B, S, D)
